# revision 13
# baseline (speedup 1.0000x reference)
"""Sliding-window GQA attention on 8 TRN2 NeuronCores (v2).

Sharding: core c handles batch b=c//4 and kv-head pair 2*(c%4)..+1
(-> 4 query heads, 2 kv heads, all 2048 tokens of one batch).
Each core computes its heads' partial o-projection [2048, 3584] in
bf16; the host sums the 4 partials per batch. No on-device collectives.

v2 structural changes vs v1:
- All transposes (q/k production, softmax probs, attention output) run
  on the DMA XBAR (dma_start_transpose) instead of PE identity-matmuls:
  ~40us of PE work + all PSUM->SBUF transpose copies removed.
- KV projection is fused with attention (lagged one 128-token block):
  the small-window attention blocks hide their vector/act latency under
  the KV projection matmul stream.
- The o-projection runs as a dense tail phase from a persistent
  transposed-output buffer; w_o streams in dx-major slabs.
- QK-norm scales are folded into the RoPE tables host-side (tables in
  bf16); output partials are written in bf16.
- w_q streams in 7 fine-grained pieces and the first two token blocks
  interleave per-piece so PE starts ~4us into the kernel.
"""

import numpy as np
import ml_dtypes

B, T, D, H = 2, 2048, 3584, 256
QH, KVH = 4, 2          # per-core q heads / kv heads
DC = D // 128           # 28 contract chunks
TBN = T // 128          # 16 token blocks
HC = H // 128           # 2 head-dim chunks
OC = QH * H // 128      # 8 out-proj contract chunks
SCALE = 0.0625
EPS = 1e-6
ROPE_BASE = 10000.0
WB = 1024 // 128        # window in blocks (8)
NEG = -1.0e30
NDOUT = D // 512        # 7 o-proj column chunks
DCQ = DC // 4           # 7 chunks per wkv quarter

BF16 = ml_dtypes.bfloat16

_cached = {}


def _build(apply_scales):
    import concourse.bass as bass
    import concourse.mybir as mybir
    import concourse.tile as tile
    from concourse import bacc

    f32 = mybir.dt.float32
    bf16 = mybir.dt.bfloat16
    AF = mybir.ActivationFunctionType

    nc = bacc.Bacc(None, target_bir_lowering=False)

    xT_d = nc.dram_tensor("xT", [128, DC, T], bf16, kind="ExternalInput")
    wq_d = nc.dram_tensor("wq", [128, DC, QH * H], bf16, kind="ExternalInput")
    wkv_d = nc.dram_tensor("wkv", [128, DC, 2 * KVH * H], bf16,
                           kind="ExternalInput")
    # dx-major o-proj weights: [128, dx, c, 512]
    wo_d = nc.dram_tensor("wo", [128, NDOUT, OC, 512], bf16,
                          kind="ExternalInput")
    # shared rope tables: [128(t), TBN, 128(h)]
    cos_d = nc.dram_tensor("cost", [128, TBN, 128], bf16, kind="ExternalInput")
    sin_d = nc.dram_tensor("sint", [128, TBN, 128], bf16, kind="ExternalInput")
    if apply_scales:
        qscT_d = nc.dram_tensor("qscT", [128, H], f32, kind="ExternalInput")
        kscT_d = nc.dram_tensor("kscT", [128, H], f32, kind="ExternalInput")
    mdiag_d = nc.dram_tensor("mdiag", [128, 128], f32, kind="ExternalInput")
    medge_d = nc.dram_tensor("medge", [128, 128], f32, kind="ExternalInput")
    out_d = nc.dram_tensor("out", [T, D], bf16, kind="ExternalOutput")

    with tile.TileContext(nc) as tc:
        with (
            tc.tile_pool(name="persist", bufs=1) as pers,
            tc.tile_pool(name="outTp", bufs=1) as poT2,
        ):
            qT = pers.tile([128, QH, HC, T], bf16)    # q^T [h, j, hc, t]
            kT = pers.tile([128, KVH, HC, T], bf16)   # k^T [h, kv, hc, s]
            vS = pers.tile([128, TBN, KVH, H], bf16)  # v   [s, sblock, kv, h]
            rqs = pers.tile([128, TBN, QH], f32)      # SCALE/rms(q)
            epsb = pers.tile([128, 1], f32)
            nc.gpsimd.memset(epsb, EPS)
            epsb2 = pers.tile([128, 1], f32)
            nc.gpsimd.memset(epsb2, EPS / (SCALE * SCALE))
            outT = poT2.tile([128, OC, TBN, 128], bf16)

            with (
                tc.tile_pool(name="tabp", bufs=1) as ptab,
                tc.tile_pool(name="wkvp", bufs=1) as wkvp,
            ):
                # quarters 0/1 live here (DMAed during the 1a tail);
                # quarters 2/3 come from a fused-phase pool to keep 1a
                # under the SBUF budget.
                wkv_p = [wkvp.tile([128, DCQ, 2 * KVH * H], bf16,
                                   tag=f"wkv{qi}", name=f"wkv_{qi}")
                         for qi in range(2)]

                # ---------------- phase 1a: Q projection ----------------
                with (
                    tc.tile_pool(name="wqp", bufs=7) as wqp,
                    tc.tile_pool(name="xt1", bufs=2) as pxt,
                    tc.tile_pool(name="scr1", bufs=2) as scr,
                    tc.tile_pool(name="qr", bufs=1) as pqr,
                    tc.tile_pool(name="ppq", bufs=6,
                                 space=bass.MemorySpace.PSUM) as ppq,
                    tc.tile_pool(name="sqp", bufs=2,
                                 space=bass.MemorySpace.PSUM) as sqp,
                ):
                    # x blocks 0/1 first on the gpsimd queue, then wq pieces
                    # on the sync queue land every ~3us; tb0/tb1 interleave
                    # per piece so PE starts once piece 0 + xt0 arrive.
                    xts0 = []
                    for tbe in range(2):
                        xte = pxt.tile([128, DC, 128], bf16, tag="xt",
                                       name="xt0")
                        nc.gpsimd.dma_start(
                            xte, xT_d[:, :, tbe * 128:(tbe + 1) * 128])
                        xts0.append(xte)
                    WP = 7  # wq pieces, 4 chunks each
                    wq_p = []
                    for p in range(WP):
                        wt = wqp.tile([128, 4, QH * H], bf16, tag="wq",
                                      name=f"wq_{p}")
                        nc.sync.dma_start(wt, wq_d[:, 4 * p:4 * p + 4, :])
                        wq_p.append(wt)
                    cost = ptab.tile([128, TBN, 128], bf16, tag="cos")
                    sint = ptab.tile([128, TBN, 128], bf16, tag="sin")
                    nc.gpsimd.dma_start(cost, cos_d[:])
                    nc.gpsimd.dma_start(sint, sin_d[:])
                    if apply_scales:
                        qscT = ptab.tile([128, H], f32, tag="qsc")
                        kscT = ptab.tile([128, H], f32, tag="ksc")
                        nc.gpsimd.dma_start(qscT, qscT_d[:])
                        nc.gpsimd.dma_start(kscT, kscT_d[:])

                    def q_epilogue(pq2, tb, qr4):
                        """norm+rope QH heads of psum pair -> qr4 -> qT."""
                        for j in range(QH):
                            pq = pq2[j // 2]
                            jj = j % 2
                            sq = sqp.tile([128, H], f32, tag="sq")
                            ssq = scr.tile([128, 1], f32, tag="ssq")
                            nc.scalar.activation(sq, pq[:, jj, :], AF.Square,
                                                 accum_out=ssq)
                            std = scr.tile([128, 1], f32, tag="std")
                            nc.scalar.activation(
                                std, ssq, AF.Sqrt, bias=epsb2[:, 0:1],
                                scale=1.0 / (H * SCALE * SCALE))
                            nc.vector.reciprocal(rqs[:, tb, j, None], std)
                            x1 = pq[:, jj, 0:128]
                            x2 = pq[:, jj, 128:256]
                            t1 = scr.tile([128, 128], f32, tag="t1")
                            t2 = scr.tile([128, 128], f32, tag="t2")
                            h1 = qr4[:, j * H:j * H + 128]
                            h2 = qr4[:, j * H + 128:(j + 1) * H]
                            nc.vector.tensor_mul(t1, x1, cost[:, tb, :])
                            nc.vector.tensor_mul(t2, x2, sint[:, tb, :])
                            if apply_scales:
                                nc.vector.tensor_sub(t1, t1, t2)
                                nc.vector.tensor_mul(h1, t1, qscT[:, 0:128])
                            else:
                                nc.vector.tensor_sub(h1, t1, t2)
                            nc.vector.tensor_mul(t1, x2, cost[:, tb, :])
                            nc.vector.tensor_mul(t2, x1, sint[:, tb, :])
                            if apply_scales:
                                nc.vector.tensor_add(t1, t1, t2)
                                nc.vector.tensor_mul(h2, t1, qscT[:, 128:256])
                            else:
                                nc.vector.tensor_add(h2, t1, t2)
                        nc.sync.dma_start_transpose(
                            qT[:, :, :, tb * 128:(tb + 1) * 128], qr4)

                    # tb 0/1 interleaved per weight piece
                    pqs01 = []
                    for tbx in range(2):
                        pqa = ppq.tile([128, 2, H], f32, tag="pq", name="pqa")
                        pqb = ppq.tile([128, 2, H], f32, tag="pq", name="pqb")
                        pqs01.append((pqa, pqb))
                    for p in range(WP):
                        for tbx in range(2):
                            pqa, pqb = pqs01[tbx]
                            for dcl in range(4):
                                lhsT = xts0[tbx][:, 4 * p + dcl, :]
                                st = (p == 0 and dcl == 0)
                                sp = (p == WP - 1 and dcl == 3)
                                nc.tensor.matmul(pqa[:, :, :], lhsT,
                                                 wq_p[p][:, dcl, 0:512],
                                                 start=st, stop=sp)
                                nc.tensor.matmul(pqb[:, :, :], lhsT,
                                                 wq_p[p][:, dcl, 512:1024],
                                                 start=st, stop=sp)
                    for tbx in range(2):
                        qr4 = pqr.tile([128, QH * H], bf16, tag="qr4")
                        q_epilogue(pqs01[tbx], tbx, qr4)

                    for tb in range(2, TBN):
                        xt = pxt.tile([128, DC, 128], bf16, tag="xt")
                        nc.gpsimd.dma_start(
                            xt, xT_d[:, :, tb * 128:(tb + 1) * 128])
                        pqa = ppq.tile([128, 2, H], f32, tag="pq", name="pqa")
                        pqb = ppq.tile([128, 2, H], f32, tag="pq", name="pqb")
                        for dc in range(DC):
                            lhsT = xt[:, dc, :]
                            nc.tensor.matmul(pqa[:, :, :], lhsT,
                                             wq_p[dc // 4][:, dc % 4, 0:512],
                                             start=(dc == 0),
                                             stop=(dc == DC - 1))
                            nc.tensor.matmul(pqb[:, :, :], lhsT,
                                             wq_p[dc // 4][:, dc % 4, 512:1024],
                                             start=(dc == 0),
                                             stop=(dc == DC - 1))
                        qr4 = pqr.tile([128, QH * H], bf16, tag="qr4")
                        q_epilogue((pqa, pqb), tb, qr4)
                        # stage 2 of 4 wkv quarters into the tail of 1a
                        if tb in (8, 12):
                            qi = (tb - 8) // 4
                            nc.sync.dma_start(
                                wkv_p[qi],
                                wkv_d[:, qi * DCQ:(qi + 1) * DCQ, :])

                # ---------- fused phase: KV projection + attention ----------
                with (
                    tc.tile_pool(name="wkvp2", bufs=1) as wkvp2,
                    tc.tile_pool(name="xt2", bufs=2) as pxt2,
                    tc.tile_pool(name="scr2", bufs=2) as scr2,
                    tc.tile_pool(name="kr", bufs=2) as pkr,
                    tc.tile_pool(name="att", bufs=2) as att,
                    tc.tile_pool(name="pcp", bufs=2) as pcp,
                    tc.tile_pool(name="pTp", bufs=2) as pTp,
                    tc.tile_pool(name="ppk", bufs=4,
                                 space=bass.MemorySpace.PSUM) as ppk,
                    tc.tile_pool(name="pl", bufs=2,
                                 space=bass.MemorySpace.PSUM) as plp,
                    tc.tile_pool(name="po", bufs=2,
                                 space=bass.MemorySpace.PSUM) as pop,
                ):
                    for qi in (2, 3):
                        wt = wkvp2.tile([128, DCQ, 2 * KVH * H], bf16,
                                        tag=f"wkv{qi}", name=f"wkv_{qi}")
                        nc.sync.dma_start(
                            wt, wkv_d[:, qi * DCQ:(qi + 1) * DCQ, :])
                        wkv_p.append(wt)
                    mdiag = att.tile([128, 128], f32, tag="mdiag", bufs=1)
                    medge = att.tile([128, 128], f32, tag="medge", bufs=1)
                    nc.sync.dma_start(mdiag, mdiag_d[:])
                    nc.sync.dma_start(medge, medge_d[:])

                    for tb in range(TBN + 1):
                        a = tb - 1
                        # -- attention(a) part 1: QK + exp + probs^T --
                        if a >= 0:
                            sb0 = max(0, a - WB)
                            ns = a - sb0 + 1
                            sw = ns * 128
                            dg = (a - sb0) * 128
                            pcs = []   # (se chunks, pT tile)
                            for j in range(QH):
                                kv = j // 2
                                pc = pcp.tile([128, 1152], bf16, tag="pc",
                                              name="pc")
                                sumes = []
                                for c0 in range(0, sw, 512):
                                    cols = min(512, sw - c0)
                                    pl = plp.tile([128, 512], f32, tag="pl",
                                                  name="pl")
                                    for hc in range(HC):
                                        nc.tensor.matmul(
                                            pl[:, :cols],
                                            qT[:, j, hc,
                                               a * 128:(a + 1) * 128],
                                            kT[:, kv, hc,
                                               sb0 * 128 + c0:
                                               sb0 * 128 + c0 + cols],
                                            start=(hc == 0),
                                            stop=(hc == HC - 1))
                                    if a >= WB and c0 == 0:
                                        nc.vector.tensor_add(
                                            pl[:, 0:128], pl[:, 0:128], medge)
                                    if c0 <= dg < c0 + cols:
                                        off = dg - c0
                                        nc.vector.tensor_add(
                                            pl[:, off:off + 128],
                                            pl[:, off:off + 128], mdiag)
                                    se = att.tile([128, 1], f32, tag="sume",
                                                  bufs=16, name="se")
                                    nc.scalar.activation(
                                        pc[:, c0:c0 + cols], pl[:, :cols],
                                        AF.Exp, scale=rqs[:, a, j, None],
                                        accum_out=se)
                                    sumes.append(se)
                                pT = pTp.tile([128, 9, 128], bf16, tag="pT",
                                              name="pT")
                                nc.sync.dma_start_transpose(
                                    pT[:, :ns, :], pc[:, :sw])
                                pcs.append((sumes, pT))

                        # -- KV projection block tb --
                        if tb < TBN:
                            xt = pxt2.tile([128, DC, 128], bf16, tag="xt")
                            nc.gpsimd.dma_start(
                                xt, xT_d[:, :, tb * 128:(tb + 1) * 128])
                            pk = ppk.tile([128, 2, H], f32, tag="pk",
                                          name="pk")
                            pv = ppk.tile([128, 2, H], f32, tag="pk",
                                          name="pv")
                            for dc in range(DC):
                                lhsT = xt[:, dc, :]
                                wt = wkv_p[dc // DCQ]
                                dcl = dc % DCQ
                                nc.tensor.matmul(pk[:, :, :], lhsT,
                                                 wt[:, dcl, 0:512],
                                                 start=(dc == 0),
                                                 stop=(dc == DC - 1))
                                nc.tensor.matmul(pv[:, :, :], lhsT,
                                                 wt[:, dcl, 512:1024],
                                                 start=(dc == 0),
                                                 stop=(dc == DC - 1))
                            # K epilogue: norm + rope -> kr4 -> kT
                            kr4 = pkr.tile([128, KVH * H], bf16, tag="kr4")
                            for kv in range(KVH):
                                sqk = scr2.tile([128, H], f32, tag="sqk")
                                ssq = scr2.tile([128, 1], f32, tag="ssqk")
                                nc.scalar.activation(sqk, pk[:, kv, :],
                                                     AF.Square,
                                                     accum_out=ssq)
                                std = scr2.tile([128, 1], f32, tag="stdk")
                                nc.scalar.activation(std, ssq, AF.Sqrt,
                                                     bias=epsb[:, 0:1],
                                                     scale=1.0 / H)
                                rstd = scr2.tile([128, 1], f32, tag="rstdk")
                                nc.vector.reciprocal(rstd, std)
                                rb = rstd[:, 0:1].to_broadcast((128, 128))
                                x1 = pk[:, kv, 0:128]
                                x2 = pk[:, kv, 128:256]
                                t1 = scr2.tile([128, 128], f32, tag="t1k")
                                t2 = scr2.tile([128, 128], f32, tag="t2k")
                                g1 = kr4[:, kv * H:kv * H + 128]
                                g2 = kr4[:, kv * H + 128:(kv + 1) * H]
                                nc.vector.tensor_mul(t1, x1, cost[:, tb, :])
                                nc.vector.tensor_mul(t2, x2, sint[:, tb, :])
                                nc.vector.tensor_sub(t1, t1, t2)
                                if apply_scales:
                                    nc.vector.tensor_mul(t1, t1, rb)
                                    nc.vector.tensor_mul(g1, t1,
                                                         kscT[:, 0:128])
                                else:
                                    nc.vector.tensor_mul(g1, t1, rb)
                                nc.vector.tensor_mul(t1, x2, cost[:, tb, :])
                                nc.vector.tensor_mul(t2, x1, sint[:, tb, :])
                                nc.vector.tensor_add(t1, t1, t2)
                                if apply_scales:
                                    nc.vector.tensor_mul(t1, t1, rb)
                                    nc.vector.tensor_mul(g2, t1,
                                                         kscT[:, 128:256])
                                else:
                                    nc.vector.tensor_mul(g2, t1, rb)
                                nc.vector.tensor_copy(vS[:, tb, kv, :],
                                                      pv[:, kv, :])
                            nc.sync.dma_start_transpose(
                                kT[:, :, :, tb * 128:(tb + 1) * 128], kr4)

                        # -- attention(a) part 2: AV + normalize + out^T --
                        if a >= 0:
                            outsb = att.tile([128, QH, H], bf16, tag="outsb")
                            for j in range(QH):
                                kv = j // 2
                                sumes, pT = pcs[j]
                                tot = sumes[0]
                                for se in sumes[1:]:
                                    t2s = att.tile([128, 1], f32,
                                                   tag="sume2", bufs=10,
                                                   name="se2")
                                    nc.vector.tensor_add(t2s, tot, se)
                                    tot = t2s
                                recip = att.tile([128, 1], f32, tag="recip",
                                                 bufs=4)
                                nc.vector.reciprocal(recip, tot)
                                po = pop.tile([128, H], f32, tag="po")
                                for s in range(ns):
                                    nc.tensor.matmul(po, pT[:, s, :],
                                                     vS[:, sb0 + s, kv, :],
                                                     start=(s == 0),
                                                     stop=(s == ns - 1))
                                nc.vector.tensor_mul(
                                    outsb[:, j, :], po,
                                    recip[:, 0:1].to_broadcast((128, H)))
                            nc.sync.dma_start_transpose(
                                outT[:, :, a, :], outsb[:, :, :])

            # ---------------- phase 3: o-projection ----------------
            with (
                tc.tile_pool(name="wop", bufs=7) as wop,
                tc.tile_pool(name="ysb", bufs=2) as pys,
                tc.tile_pool(name="py", bufs=4,
                             space=bass.MemorySpace.PSUM) as pyp,
            ):
                wo_p = []
                for dx in range(NDOUT):
                    wt = wop.tile([128, OC, 512], bf16, tag="wo",
                                  name=f"wo_{dx}")
                    nc.sync.dma_start(wt, wo_d[:, dx, :, :])
                    wo_p.append(wt)
                for tb in range(TBN):
                    ysb = pys.tile([128, D], bf16, tag="y")
                    for dx in range(NDOUT):
                        py = pyp.tile([128, 512], f32, tag="py")
                        for c in range(OC):
                            nc.tensor.matmul(py, outT[:, c, tb, :],
                                             wo_p[dx][:, c, :],
                                             start=(c == 0),
                                             stop=(c == OC - 1))
                        if dx % 2 == 0:
                            nc.vector.tensor_copy(
                                ysb[:, dx * 512:(dx + 1) * 512], py)
                        else:
                            nc.scalar.activation(
                                ysb[:, dx * 512:(dx + 1) * 512], py, AF.Copy)
                    nc.gpsimd.dma_start(
                        out_d[tb * 128:(tb + 1) * 128, :], ysb)

    nc.compile()
    return nc


def _tile128(a):
    """[128*n, m] -> [128, n, m] with row index = chunk*128 + partition."""
    n = a.shape[0] // 128
    return np.ascontiguousarray(
        a.reshape(n, 128, *a.shape[1:]).transpose(1, 0, *range(2, a.ndim + 1)))


def _rope_tabs():
    """cos/sin tables [128, TBN, 128] bf16 (t on partitions, h free)."""
    j = np.arange(128, dtype=np.float64)
    ts = ROPE_BASE ** (2.0 * j / H)
    ang = np.arange(T, dtype=np.float64)[:, None] / ts[None, :]
    return (_tile128(np.cos(ang).astype(np.float32).astype(BF16)),
            _tile128(np.sin(ang).astype(np.float32).astype(BF16)))


def kernel(x, w_q, w_kv, w_o, q_norm_scale, k_norm_scale):
    from concourse.bass_utils import run_bass_kernel_spmd

    qsc0 = np.asarray(q_norm_scale, np.float32)
    ksc0 = np.asarray(k_norm_scale, np.float32)
    apply_scales = not (np.all(qsc0 == 1.0) and np.all(ksc0 == 1.0))
    key = ("nc", apply_scales)
    if key not in _cached:
        _cached[key] = _build(apply_scales)
    nc = _cached[key]

    x = np.asarray(x, np.float32)
    w_q = np.asarray(w_q, np.float32)
    w_kv = np.asarray(w_kv, np.float32)
    w_o = np.asarray(w_o, np.float32)
    qsc = np.asarray(q_norm_scale, np.float32)
    ksc = np.asarray(k_norm_scale, np.float32)
    cos_t, sin_t = _rope_tabs()

    p = np.arange(128)[:, None]
    f = np.arange(128)[None, :]
    mdiag = np.where(p >= f, 0.0, NEG).astype(np.float32)
    medge = np.where(f >= p + 1, 0.0, NEG).astype(np.float32)

    xT_b = []
    for b in range(B):
        xT_b.append(_tile128(np.ascontiguousarray(x[b].T).astype(BF16)))

    in_maps = []
    for c in range(8):
        b, kp = c // 4, c % 4
        n0, k0 = 4 * kp, 2 * kp
        wq = _tile128(w_q[n0:n0 + 4].transpose(1, 0, 2)
                      .reshape(D, QH * H).astype(BF16))
        wk = w_kv[0, k0:k0 + 2].transpose(1, 0, 2).reshape(D, KVH * H)
        wv = w_kv[1, k0:k0 + 2].transpose(1, 0, 2).reshape(D, KVH * H)
        wkv = _tile128(np.concatenate([wk, wv], axis=1).astype(BF16))
        wo_t = _tile128(w_o[n0:n0 + 4].reshape(QH * H, D).astype(BF16))
        # [128, OC, D] -> [128, NDOUT, OC, 512]
        wo2 = np.ascontiguousarray(
            wo_t.reshape(128, OC, NDOUT, 512).transpose(0, 2, 1, 3))
        m = {"xT": xT_b[b], "wq": wq, "wkv": wkv, "wo": wo2,
             "mdiag": mdiag, "medge": medge,
             "cost": cos_t, "sint": sin_t}
        if apply_scales:
            m["qscT"] = np.broadcast_to(qsc, (128, H)).copy()
            m["kscT"] = np.broadcast_to(ksc, (128, H)).copy()
        in_maps.append(m)

    res = run_bass_kernel_spmd(nc, in_maps, core_ids=list(range(8)))
    _cached["last_result"] = res
    y = np.zeros((B, T, D), np.float32)
    for c in range(8):
        y[c // 4] += np.asarray(res.results[c]["out"], np.float32)
    return y


# revision 16
# speedup vs baseline: 1.0537x; 1.0537x over previous
"""Sliding-window GQA attention on 8 TRN2 NeuronCores (v2).

Sharding: core c handles batch b=c//4 and kv-head pair 2*(c%4)..+1
(-> 4 query heads, 2 kv heads, all 2048 tokens of one batch).
Each core computes its heads' partial o-projection [2048, 3584] in
bf16; the host sums the 4 partials per batch. No on-device collectives.

v2 structural changes vs v1:
- All transposes (q/k production, softmax probs, attention output) run
  on the DMA XBAR (dma_start_transpose) instead of PE identity-matmuls:
  ~40us of PE work + all PSUM->SBUF transpose copies removed.
- KV projection is fused with attention (lagged one 128-token block):
  the small-window attention blocks hide their vector/act latency under
  the KV projection matmul stream.
- The o-projection runs as a dense tail phase from a persistent
  transposed-output buffer; w_o streams in dx-major slabs.
- QK-norm scales are folded into the RoPE tables host-side (tables in
  bf16); output partials are written in bf16.
- w_q streams in 7 fine-grained pieces and the first two token blocks
  interleave per-piece so PE starts ~4us into the kernel.
"""

import numpy as np
import ml_dtypes

B, T, D, H = 2, 2048, 3584, 256
QH, KVH = 4, 2          # per-core q heads / kv heads
DC = D // 128           # 28 contract chunks
TBN = T // 128          # 16 token blocks
HC = H // 128           # 2 head-dim chunks
OC = QH * H // 128      # 8 out-proj contract chunks
SCALE = 0.0625
EPS = 1e-6
ROPE_BASE = 10000.0
WB = 1024 // 128        # window in blocks (8)
NEG = -1.0e30
NDOUT = D // 512        # 7 o-proj column chunks
LOG_SCALE = float(np.log(SCALE))
DCQ = DC // 4           # 7 chunks per wkv quarter

BF16 = ml_dtypes.bfloat16

_cached = {}


def _build(apply_scales):
    import concourse.bass as bass
    import concourse.mybir as mybir
    import concourse.tile as tile
    from concourse import bacc

    f32 = mybir.dt.float32
    bf16 = mybir.dt.bfloat16
    AF = mybir.ActivationFunctionType

    nc = bacc.Bacc(None, target_bir_lowering=False)

    xB_d = nc.dram_tensor("xB", [TBN, 128, DC, 128], bf16,
                      kind="ExternalInput")
    wq_d = nc.dram_tensor("wq", [128, DC, QH * H], bf16, kind="ExternalInput")
    wkv_d = nc.dram_tensor("wkv", [128, DC, 2 * KVH * H], bf16,
                           kind="ExternalInput")
    # dx-major o-proj weights: [128, dx, c, 512]
    wo_d = nc.dram_tensor("wo", [128, NDOUT, OC, 512], bf16,
                          kind="ExternalInput")
    # shared rope tables: [128(t), TBN, 128(h)]
    cos_d = nc.dram_tensor("cost", [128, TBN, 128], bf16, kind="ExternalInput")
    sin_d = nc.dram_tensor("sint", [128, TBN, 128], bf16, kind="ExternalInput")
    if apply_scales:
        qscT_d = nc.dram_tensor("qscT", [128, H], f32, kind="ExternalInput")
        kscT_d = nc.dram_tensor("kscT", [128, H], f32, kind="ExternalInput")
    mdiag_d = nc.dram_tensor("mdiag", [128, 128], f32, kind="ExternalInput")
    medge_d = nc.dram_tensor("medge", [128, 128], f32, kind="ExternalInput")
    out_d = nc.dram_tensor("out", [T, D], bf16, kind="ExternalOutput")

    with tile.TileContext(nc) as tc:
        with (
            tc.tile_pool(name="persist", bufs=1) as pers,
            tc.tile_pool(name="outTp", bufs=1) as poT2,
        ):
            qT = pers.tile([128, QH, HC, T], bf16)    # q^T [h, j, hc, t]
            kT = pers.tile([128, KVH, HC, T], bf16)   # k^T [h, kv, hc, s]
            vS = pers.tile([128, TBN, KVH, H], bf16)  # v   [s, sblock, kv, h]
            rqs = pers.tile([128, TBN, QH], f32)      # SCALE/rms(q)
            epsb = pers.tile([128, 1], f32)
            nc.gpsimd.memset(epsb, EPS)
            logsb = pers.tile([128, 1], f32)
            nc.gpsimd.memset(logsb, LOG_SCALE)
            outT = poT2.tile([128, OC, TBN, 128], bf16)

            with (
                tc.tile_pool(name="tabp", bufs=1) as ptab,
                tc.tile_pool(name="wkvp", bufs=1) as wkvp,
            ):
                # quarters 0/1 live here (DMAed during the 1a tail);
                # quarters 2/3 come from a fused-phase pool to keep 1a
                # under the SBUF budget.
                wkv_p = [wkvp.tile([128, DCQ, 2 * KVH * H], bf16,
                                   tag=f"wkv{qi}", name=f"wkv_{qi}")
                         for qi in range(2)]

                # ---------------- phase 1a: Q projection ----------------
                with (
                    tc.tile_pool(name="wqp", bufs=7) as wqp,
                    tc.tile_pool(name="xt1", bufs=2) as pxt,
                    tc.tile_pool(name="scr1", bufs=2) as scr,
                    tc.tile_pool(name="qr", bufs=1) as pqr,
                    tc.tile_pool(name="ppq", bufs=6,
                                 space=bass.MemorySpace.PSUM) as ppq,
                    tc.tile_pool(name="sqp", bufs=2,
                                 space=bass.MemorySpace.PSUM) as sqp,
                ):
                    # x blocks 0/1 first on the gpsimd queue, then wq pieces
                    # on the sync queue land every ~3us; tb0/tb1 interleave
                    # per piece so PE starts once piece 0 + xt0 arrive.
                    xts0 = []
                    for tbe in range(2):
                        xte = pxt.tile([128, DC, 128], bf16, tag="xt",
                                       name="xt0")
                        nc.gpsimd.dma_start(xte, xB_d[tbe])
                        xts0.append(xte)
                    WP = 7  # wq pieces, 4 chunks each
                    wq_p = []
                    for p in range(WP):
                        wt = wqp.tile([128, 4, QH * H], bf16, tag="wq",
                                      name=f"wq_{p}")
                        nc.sync.dma_start(wt, wq_d[:, 4 * p:4 * p + 4, :])
                        wq_p.append(wt)
                    cost = ptab.tile([128, TBN, 128], bf16, tag="cos")
                    sint = ptab.tile([128, TBN, 128], bf16, tag="sin")
                    nc.gpsimd.dma_start(cost, cos_d[:])
                    nc.gpsimd.dma_start(sint, sin_d[:])
                    if apply_scales:
                        qscT = ptab.tile([128, H], f32, tag="qsc")
                        kscT = ptab.tile([128, H], f32, tag="ksc")
                        nc.gpsimd.dma_start(qscT, qscT_d[:])
                        nc.gpsimd.dma_start(kscT, kscT_d[:])

                    def q_epilogue(pq2, tb, qr4):
                        """norm+rope QH heads of psum pair -> qr4 -> qT."""
                        for j in range(QH):
                            pq = pq2[j // 2]
                            jj = j % 2
                            sq = sqp.tile([128, H], f32, tag="sq")
                            ssq = scr.tile([128, 1], f32, tag="ssq")
                            nc.scalar.activation(sq, pq[:, jj, :], AF.Square,
                                                 accum_out=ssq)
                            lns = scr.tile([128, 1], f32, tag="lns")
                            nc.scalar.activation(lns, ssq, AF.Ln,
                                                 bias=epsb[:, 0:1],
                                                 scale=1.0 / H)
                            nc.scalar.activation(rqs[:, tb, j, None], lns,
                                                 AF.Exp, scale=-0.5,
                                                 bias=logsb[:, 0:1])
                            x1 = pq[:, jj, 0:128]
                            x2 = pq[:, jj, 128:256]
                            t1 = scr.tile([128, 128], f32, tag="t1")
                            t2 = scr.tile([128, 128], f32, tag="t2")
                            h1 = qr4[:, j * H:j * H + 128]
                            h2 = qr4[:, j * H + 128:(j + 1) * H]
                            nc.vector.tensor_mul(t1, x1, cost[:, tb, :])
                            nc.vector.tensor_mul(t2, x2, sint[:, tb, :])
                            if apply_scales:
                                nc.vector.tensor_sub(t1, t1, t2)
                                nc.vector.tensor_mul(h1, t1, qscT[:, 0:128])
                            else:
                                nc.vector.tensor_sub(h1, t1, t2)
                            nc.vector.tensor_mul(t1, x2, cost[:, tb, :])
                            nc.vector.tensor_mul(t2, x1, sint[:, tb, :])
                            if apply_scales:
                                nc.vector.tensor_add(t1, t1, t2)
                                nc.vector.tensor_mul(h2, t1, qscT[:, 128:256])
                            else:
                                nc.vector.tensor_add(h2, t1, t2)
                        nc.sync.dma_start_transpose(
                            qT[:, :, :, tb * 128:(tb + 1) * 128], qr4)

                    # tb 0/1 interleaved per weight piece
                    pqs01 = []
                    for tbx in range(2):
                        pqa = ppq.tile([128, 2, H], f32, tag="pq", name="pqa")
                        pqb = ppq.tile([128, 2, H], f32, tag="pq", name="pqb")
                        pqs01.append((pqa, pqb))
                    for p in range(WP):
                        for tbx in range(2):
                            pqa, pqb = pqs01[tbx]
                            for dcl in range(4):
                                lhsT = xts0[tbx][:, 4 * p + dcl, :]
                                st = (p == 0 and dcl == 0)
                                sp = (p == WP - 1 and dcl == 3)
                                nc.tensor.matmul(pqa[:, :, :], lhsT,
                                                 wq_p[p][:, dcl, 0:512],
                                                 start=st, stop=sp)
                                nc.tensor.matmul(pqb[:, :, :], lhsT,
                                                 wq_p[p][:, dcl, 512:1024],
                                                 start=st, stop=sp)
                    for tbx in range(2):
                        qr4 = pqr.tile([128, QH * H], bf16, tag="qr4")
                        q_epilogue(pqs01[tbx], tbx, qr4)

                    for tb in range(2, TBN):
                        xt = pxt.tile([128, DC, 128], bf16, tag="xt")
                        nc.gpsimd.dma_start(xt, xB_d[tb])
                        pqa = ppq.tile([128, 2, H], f32, tag="pq", name="pqa")
                        pqb = ppq.tile([128, 2, H], f32, tag="pq", name="pqb")
                        for dc in range(DC):
                            lhsT = xt[:, dc, :]
                            nc.tensor.matmul(pqa[:, :, :], lhsT,
                                             wq_p[dc // 4][:, dc % 4, 0:512],
                                             start=(dc == 0),
                                             stop=(dc == DC - 1))
                            nc.tensor.matmul(pqb[:, :, :], lhsT,
                                             wq_p[dc // 4][:, dc % 4, 512:1024],
                                             start=(dc == 0),
                                             stop=(dc == DC - 1))
                        qr4 = pqr.tile([128, QH * H], bf16, tag="qr4")
                        q_epilogue((pqa, pqb), tb, qr4)
                        # stage 2 of 4 wkv quarters into the tail of 1a
                        if tb in (8, 12):
                            qi = (tb - 8) // 4
                            nc.sync.dma_start(
                                wkv_p[qi],
                                wkv_d[:, qi * DCQ:(qi + 1) * DCQ, :])

                # ---------- fused phase: KV projection + attention ----------
                with (
                    tc.tile_pool(name="wkvp2", bufs=1) as wkvp2,
                    tc.tile_pool(name="xt2", bufs=2) as pxt2,
                    tc.tile_pool(name="scr2", bufs=2) as scr2,
                    tc.tile_pool(name="kr", bufs=2) as pkr,
                    tc.tile_pool(name="att", bufs=2) as att,
                    tc.tile_pool(name="pcp", bufs=1) as pcp,
                    tc.tile_pool(name="pTp", bufs=1) as pTp,
                    tc.tile_pool(name="ppk", bufs=4,
                                 space=bass.MemorySpace.PSUM) as ppk,
                    tc.tile_pool(name="pl", bufs=2,
                                 space=bass.MemorySpace.PSUM) as plp,
                    tc.tile_pool(name="po", bufs=2,
                                 space=bass.MemorySpace.PSUM) as pop,
                ):
                    xts2 = []
                    for tbe in range(2):
                        xte = pxt2.tile([128, DC, 128], bf16, tag="xt",
                                        name="xt2p")
                        nc.gpsimd.dma_start(xte, xB_d[tbe])
                        xts2.append(xte)
                    for qi, eng in ((2, nc.sync), (3, nc.gpsimd)):
                        wt = wkvp2.tile([128, DCQ, 2 * KVH * H], bf16,
                                        tag=f"wkv{qi}", name=f"wkv_{qi}")
                        eng.dma_start(
                            wt, wkv_d[:, qi * DCQ:(qi + 1) * DCQ, :])
                        wkv_p.append(wt)
                    pc_all = pcp.tile([128, QH, 1152], bf16, tag="pc",
                                      name="pc")
                    nc.gpsimd.memset(pc_all, 0.0)
                    pT_all = pTp.tile([128, QH, 9, 128], bf16, tag="pT",
                                      name="pT")
                    mdiag = att.tile([128, 128], f32, tag="mdiag", bufs=1)
                    medge = att.tile([128, 128], f32, tag="medge", bufs=1)
                    nc.sync.dma_start(mdiag, mdiag_d[:])
                    nc.sync.dma_start(medge, medge_d[:])

                    for tb in range(TBN + 1):
                        a = tb - 1
                        # -- attention(a) part 1: QK + exp + probs^T --
                        if a >= 0:
                            sb0 = max(0, a - WB)
                            ns = a - sb0 + 1
                            sw = ns * 128
                            dg = (a - sb0) * 128
                            pcs = []   # se chunks per j
                            for j in range(QH):
                                kv = j // 2
                                pc = pc_all[:, j, :]
                                sumes = []
                                for c0 in range(0, sw, 512):
                                    cols = min(512, sw - c0)
                                    pl = plp.tile([128, 512], f32, tag="pl",
                                                  name="pl")
                                    for hc in range(HC):
                                        nc.tensor.matmul(
                                            pl[:, :cols],
                                            qT[:, j, hc,
                                               a * 128:(a + 1) * 128],
                                            kT[:, kv, hc,
                                               sb0 * 128 + c0:
                                               sb0 * 128 + c0 + cols],
                                            start=(hc == 0),
                                            stop=(hc == HC - 1))
                                    if a >= WB and c0 == 0:
                                        nc.vector.tensor_add(
                                            pl[:, 0:128], pl[:, 0:128], medge)
                                    if c0 <= dg < c0 + cols:
                                        off = dg - c0
                                        nc.vector.tensor_add(
                                            pl[:, off:off + 128],
                                            pl[:, off:off + 128], mdiag)
                                    se = att.tile([128, 1], f32, tag="sume",
                                                  bufs=16, name="se")
                                    nc.scalar.activation(
                                        pc[:, c0:c0 + cols], pl[:, :cols],
                                        AF.Exp, scale=rqs[:, a, j, None],
                                        accum_out=se)
                                    sumes.append(se)
                                pcs.append(sumes)
                            nc.sync.dma_start_transpose(
                                pT_all[:, :, :, :], pc_all[:, :, :])

                        # -- KV projection block tb --
                        if tb < TBN:
                            if tb < 2:
                                xt = xts2[tb]
                            else:
                                xt = pxt2.tile([128, DC, 128], bf16, tag="xt")
                                nc.gpsimd.dma_start(xt, xB_d[tb])
                            pk = ppk.tile([128, 2, H], f32, tag="pk",
                                          name="pk")
                            pv = ppk.tile([128, 2, H], f32, tag="pk",
                                          name="pv")
                            for dc in range(DC):
                                lhsT = xt[:, dc, :]
                                wt = wkv_p[dc // DCQ]
                                dcl = dc % DCQ
                                nc.tensor.matmul(pk[:, :, :], lhsT,
                                                 wt[:, dcl, 0:512],
                                                 start=(dc == 0),
                                                 stop=(dc == DC - 1))
                                nc.tensor.matmul(pv[:, :, :], lhsT,
                                                 wt[:, dcl, 512:1024],
                                                 start=(dc == 0),
                                                 stop=(dc == DC - 1))
                            # K epilogue: norm + rope -> kr4 -> kT
                            kr4 = pkr.tile([128, KVH * H], bf16, tag="kr4")
                            for kv in range(KVH):
                                sqk = scr2.tile([128, H], f32, tag="sqk")
                                ssq = scr2.tile([128, 1], f32, tag="ssqk")
                                nc.scalar.activation(sqk, pk[:, kv, :],
                                                     AF.Square,
                                                     accum_out=ssq)
                                lns = scr2.tile([128, 1], f32, tag="lnsk")
                                nc.scalar.activation(lns, ssq, AF.Ln,
                                                     bias=epsb[:, 0:1],
                                                     scale=1.0 / H)
                                rstd = scr2.tile([128, 1], f32, tag="rstdk")
                                nc.scalar.activation(rstd, lns, AF.Exp,
                                                     scale=-0.5)
                                rb = rstd[:, 0:1].to_broadcast((128, 128))
                                x1 = pk[:, kv, 0:128]
                                x2 = pk[:, kv, 128:256]
                                t1 = scr2.tile([128, 128], f32, tag="t1k")
                                t2 = scr2.tile([128, 128], f32, tag="t2k")
                                g1 = kr4[:, kv * H:kv * H + 128]
                                g2 = kr4[:, kv * H + 128:(kv + 1) * H]
                                nc.vector.tensor_mul(t1, x1, cost[:, tb, :])
                                nc.vector.tensor_mul(t2, x2, sint[:, tb, :])
                                nc.vector.tensor_sub(t1, t1, t2)
                                if apply_scales:
                                    nc.vector.tensor_mul(t1, t1, rb)
                                    nc.vector.tensor_mul(g1, t1,
                                                         kscT[:, 0:128])
                                else:
                                    nc.vector.tensor_mul(g1, t1, rb)
                                nc.vector.tensor_mul(t1, x2, cost[:, tb, :])
                                nc.vector.tensor_mul(t2, x1, sint[:, tb, :])
                                nc.vector.tensor_add(t1, t1, t2)
                                if apply_scales:
                                    nc.vector.tensor_mul(t1, t1, rb)
                                    nc.vector.tensor_mul(g2, t1,
                                                         kscT[:, 128:256])
                                else:
                                    nc.vector.tensor_mul(g2, t1, rb)
                                nc.vector.tensor_copy(vS[:, tb, kv, :],
                                                      pv[:, kv, :])
                            nc.sync.dma_start_transpose(
                                kT[:, :, :, tb * 128:(tb + 1) * 128], kr4)

                        # -- attention(a) part 2: AV + normalize + out^T --
                        if a >= 0:
                            outsb = att.tile([128, QH, H], bf16, tag="outsb")
                            for j in range(QH):
                                kv = j // 2
                                sumes = pcs[j]
                                tot = sumes[0]
                                for se in sumes[1:]:
                                    t2s = att.tile([128, 1], f32,
                                                   tag="sume2", bufs=10,
                                                   name="se2")
                                    nc.vector.tensor_add(t2s, tot, se)
                                    tot = t2s
                                recip = att.tile([128, 1], f32, tag="recip",
                                                 bufs=4)
                                nc.vector.reciprocal(recip, tot)
                                po = pop.tile([128, H], f32, tag="po")
                                for s in range(ns):
                                    nc.tensor.matmul(po, pT_all[:, j, s, :],
                                                     vS[:, sb0 + s, kv, :],
                                                     start=(s == 0),
                                                     stop=(s == ns - 1))
                                nc.vector.tensor_mul(
                                    outsb[:, j, :], po,
                                    recip[:, 0:1].to_broadcast((128, H)))
                            nc.sync.dma_start_transpose(
                                outT[:, :, a, :], outsb[:, :, :])

            # ---------------- phase 3: o-projection ----------------
            with (
                tc.tile_pool(name="wop", bufs=7) as wop,
                tc.tile_pool(name="ysb", bufs=2) as pys,
                tc.tile_pool(name="py", bufs=4,
                             space=bass.MemorySpace.PSUM) as pyp,
            ):
                wo_p = []
                for dx in range(NDOUT):
                    wt = wop.tile([128, OC, 512], bf16, tag="wo",
                                  name=f"wo_{dx}")
                    (nc.sync if dx % 2 == 0 else nc.gpsimd).dma_start(
                        wt, wo_d[:, dx, :, :])
                    wo_p.append(wt)
                for tb in range(TBN):
                    ysb = pys.tile([128, D], bf16, tag="y")
                    for dx in range(NDOUT):
                        py = pyp.tile([128, 512], f32, tag="py")
                        for c in range(OC):
                            nc.tensor.matmul(py, outT[:, c, tb, :],
                                             wo_p[dx][:, c, :],
                                             start=(c == 0),
                                             stop=(c == OC - 1))
                        if dx % 2 == 0:
                            nc.vector.tensor_copy(
                                ysb[:, dx * 512:(dx + 1) * 512], py)
                        else:
                            nc.scalar.activation(
                                ysb[:, dx * 512:(dx + 1) * 512], py, AF.Copy)
                    nc.gpsimd.dma_start(
                        out_d[tb * 128:(tb + 1) * 128, :], ysb)

    nc.compile()
    return nc


def _tile128(a):
    """[128*n, m] -> [128, n, m] with row index = chunk*128 + partition."""
    n = a.shape[0] // 128
    return np.ascontiguousarray(
        a.reshape(n, 128, *a.shape[1:]).transpose(1, 0, *range(2, a.ndim + 1)))


def _rope_tabs():
    """cos/sin tables [128, TBN, 128] bf16 (t on partitions, h free)."""
    j = np.arange(128, dtype=np.float64)
    ts = ROPE_BASE ** (2.0 * j / H)
    ang = np.arange(T, dtype=np.float64)[:, None] / ts[None, :]
    return (_tile128(np.cos(ang).astype(np.float32).astype(BF16)),
            _tile128(np.sin(ang).astype(np.float32).astype(BF16)))


def kernel(x, w_q, w_kv, w_o, q_norm_scale, k_norm_scale):
    from concourse.bass_utils import run_bass_kernel_spmd

    qsc0 = np.asarray(q_norm_scale, np.float32)
    ksc0 = np.asarray(k_norm_scale, np.float32)
    apply_scales = not (np.all(qsc0 == 1.0) and np.all(ksc0 == 1.0))
    key = ("nc", apply_scales)
    if key not in _cached:
        _cached[key] = _build(apply_scales)
    nc = _cached[key]

    x = np.asarray(x, np.float32)
    w_q = np.asarray(w_q, np.float32)
    w_kv = np.asarray(w_kv, np.float32)
    w_o = np.asarray(w_o, np.float32)
    qsc = np.asarray(q_norm_scale, np.float32)
    ksc = np.asarray(k_norm_scale, np.float32)
    cos_t, sin_t = _rope_tabs()

    p = np.arange(128)[:, None]
    f = np.arange(128)[None, :]
    mdiag = np.where(p >= f, 0.0, NEG).astype(np.float32)
    medge = np.where(f >= p + 1, 0.0, NEG).astype(np.float32)

    xB_b = []
    for b in range(B):
        xT = _tile128(np.ascontiguousarray(x[b].T).astype(BF16))
        xB = np.ascontiguousarray(
            xT.reshape(128, DC, TBN, 128).transpose(2, 0, 1, 3))
        xB_b.append(xB)

    in_maps = []
    for c in range(8):
        b, kp = c // 4, c % 4
        n0, k0 = 4 * kp, 2 * kp
        wq = _tile128(w_q[n0:n0 + 4].transpose(1, 0, 2)
                      .reshape(D, QH * H).astype(BF16))
        wk = w_kv[0, k0:k0 + 2].transpose(1, 0, 2).reshape(D, KVH * H)
        wv = w_kv[1, k0:k0 + 2].transpose(1, 0, 2).reshape(D, KVH * H)
        wkv = _tile128(np.concatenate([wk, wv], axis=1).astype(BF16))
        wo_t = _tile128(w_o[n0:n0 + 4].reshape(QH * H, D).astype(BF16))
        # [128, OC, D] -> [128, NDOUT, OC, 512]
        wo2 = np.ascontiguousarray(
            wo_t.reshape(128, OC, NDOUT, 512).transpose(0, 2, 1, 3))
        m = {"xB": xB_b[b], "wq": wq, "wkv": wkv, "wo": wo2,
             "mdiag": mdiag, "medge": medge,
             "cost": cos_t, "sint": sin_t}
        if apply_scales:
            m["qscT"] = np.broadcast_to(qsc, (128, H)).copy()
            m["kscT"] = np.broadcast_to(ksc, (128, H)).copy()
        in_maps.append(m)

    res = run_bass_kernel_spmd(nc, in_maps, core_ids=list(range(8)))
    _cached["last_result"] = res
    y = np.zeros((B, T, D), np.float32)
    for c in range(8):
        y[c // 4] += np.asarray(res.results[c]["out"], np.float32)
    return y
